# revision 5
# baseline (speedup 1.0000x reference)
"""MoE (8 experts, top-2, cap-drop) Trainium2 kernel over 8 NeuronCores.

Strategy v2 (expert-parallel, one full expert per core):
 - Router runs replicated on host with the exact fp32 jax ops of the
   reference so top-2/capacity decisions match the oracle bit-for-bit;
   routing IS the sharding function (it decides which token rows go to
   which expert core).
 - Gate folding: FFN(x) = W2^T relu(W1^T x) is positively homogeneous, and
   gates are softmax probs > 0, so g*FFN(x) = FFN(g*x). The host scales each
   gathered token column by its gate BEFORE the bf16 cast; the device then
   runs a pure dense FFN and the output needs no gating or masking at all
   (zero-gate padding columns produce exactly 0).
 - Dispatch/shard: per expert e (= core e), gather its routed token rows,
   scale by gate, ship transposed (D on partitions) in bf16, padded to the
   shared column count CMAX (all cores must run the same SPMD program).
 - Per core: ysT = W2^T-chain( relu( W1^T-chain( xT ) ) ) over column
   blocks of 512 (tail to CMAX%512); all matmuls bf16 with fp32 PSUM.
   All weights (16.8 MB bf16) are DMA'd up front and stay SBUF-resident.
 - Combine/unshard: output ships back as bf16 [8,128,CMAX]; host
   scatter-adds the first count_e (real) columns into y in f32.

vs v1 (two half-expert units per core, f32 partial outputs): same FLOPs,
but per-core host<->device traffic drops 45.2 MB -> ~25.7 MB (no f32
partials, no half-duplication of outputs, exact-count padding).

Dual-ring prelude (v7): block-0 xg tiles stream on the SP HWDGE ring
while all weight tiles stream concurrently on the ACT HWDGE ring
(nc.scalar.dma_start), so the first matmul's operands clear the ~640ns
per-DMA ring serialization ~4us sooner; output DMAs also ride the ACT
ring. Traced: the matmul stream itself runs at 99.8% of the theoretical
issue rate (216.2ns/MM at N=512, 56.2ns/MM at N=128, LDWEIGHTS fully
hidden), so startup/tail were the only remaining slack.

Self-contained: shapes hardcoded for B=4, S=2048, D=1024, F=4096, E=8,
top-2, cap=2560, 8 cores.
"""

import sys

for _p in ("/opt/trn_rl_repo",):
    if _p not in sys.path:
        sys.path.append(_p)

import math
import os

import numpy as np
import ml_dtypes

BF16 = ml_dtypes.bfloat16

B, S, D, F, E = 4, 2048, 1024, 4096, 8
TOP_K = 2
CAP_FACTOR = 1.25
T = B * S                                   # 8192 tokens
CAP = max(math.ceil(T * TOP_K * CAP_FACTOR / E), 1)   # 2560
NCORES = 8
BLK = 512                                   # token block (matmul moving dim)
P = 128
KD = D // P                                 # 8 k-chunks (embed)
KF = F // P                                 # 32 k-chunks (ffn)


def _route(xf: np.ndarray, Wr: np.ndarray):
    """Replicate the reference's routing bit-for-bit on jax-CPU.

    Returns per-expert (idx[CAP] int64 token ids, gate[CAP] f32, 0 on padding).
    """
    import jax
    import jax.numpy as jnp

    cpu = jax.devices("cpu")[0]
    with jax.default_device(cpu):
        xj = jnp.asarray(xf, dtype=jnp.float32)
        wr = jnp.asarray(Wr, dtype=jnp.float32)
        probs = jax.nn.softmax(xj.astype(jnp.float32) @ wr, axis=-1)
        topk_probs, topk_experts = jax.lax.top_k(probs, TOP_K)
        idxs, gates = [], []
        for e in range(E):
            mask = topk_experts == e
            gate = jnp.sum(jnp.where(mask, topk_probs, 0.0), axis=-1)
            has = jnp.any(mask, axis=-1)
            g_masked = jnp.where(has, gate, -jnp.inf)
            vals, idx = jax.lax.top_k(g_masked, CAP)
            g = jnp.where(jnp.isfinite(vals), vals, 0.0)
            idxs.append(np.asarray(idx, dtype=np.int64))
            gates.append(np.asarray(g, dtype=np.float32))
    return idxs, gates


_COMPILED = {}


def _blocks_of(cmax: int):
    bs = [BLK] * (cmax // BLK)
    if cmax % BLK:
        bs.append(cmax % BLK)
    return bs


def _build(cmax: int, reps: int = 1):
    """Compile the SPMD per-core program: one dense relu-MLP (full expert)
    over cmax token columns."""
    import concourse.bacc as bacc
    import concourse.mybir as mybir
    import concourse.tile as tile

    f32 = mybir.dt.float32
    bf16 = mybir.dt.bfloat16

    blocks = _blocks_of(cmax)

    nc = bacc.Bacc("TRN2", target_bir_lowering=False, debug=False,
                   num_devices=NCORES)
    xg = nc.dram_tensor("xg", [KD, P, cmax], bf16, kind="ExternalInput")
    # w1 host-pretiled f-major: [f, p, k*P+m] = W1[e][k*P+p, f*P+m]
    w1 = nc.dram_tensor("w1", [KF, P, D], bf16, kind="ExternalInput")
    # w2 host-pretiled d-major: [d, p, k2*P+m] = W2[e][k2*P+p, d*P+m]
    w2 = nc.dram_tensor("w2", [KD, P, F], bf16, kind="ExternalInput")
    ys = nc.dram_tensor("ys", [KD, P, cmax], bf16, kind="ExternalOutput")
    warm = nc.dram_tensor("warm", [P, BLK // 2], f32, kind="ExternalOutput")

    with tile.TileContext(nc) as tc:
        with (
            tc.tile_pool(name="w1p", bufs=1) as w1p,
            tc.tile_pool(name="w2p", bufs=1) as w2p,
            tc.tile_pool(name="xg0p", bufs=1) as xg0p,
            tc.tile_pool(name="xgp", bufs=2) as xgp,
            tc.tile_pool(name="htp", bufs=1) as htp,
            tc.tile_pool(name="outp", bufs=2) as outp,
            tc.tile_pool(name="warmp", bufs=1) as warmp,
            tc.tile_pool(name="ps1", bufs=4, space="PSUM") as ps1,
            tc.tile_pool(name="ps2", bufs=4, space="PSUM") as ps2,
        ):
            # PE warm-up: dummy matmuls on a memset tile keep the HAM
            # activity monitor busy (full 2.4 GHz clock) while the first
            # real xg/W1 DMAs land; they depend on no DMA and start at t0.
            wsrc = warmp.tile([P, BLK // 2], bf16, tag="warm_src")
            nc.vector.memset(wsrc[:], 0)
            wps = ps1.tile([P, BLK // 2], f32, tag="ps")
            for r in range(12):
                nc.tensor.matmul(wps[:], wsrc[:, :P], wsrc[:],
                                 start=(r == 0), stop=(r == 11))
            wout = warmp.tile([P, BLK // 2], f32, tag="warm_out")
            nc.vector.tensor_copy(wout[:], wps[:])
            nc.scalar.dma_start(warm[:], wout[:])

            # Prelude DMA, issued once before the (optional) rep loop:
            # block-0 xg tiles first (the first matmul's operand — heads of
            # the round-robin DMA lanes), then w1 tiles (consumed first by
            # mm1, ~1.7us apart), then w2 (mm2 starts ~55us in; the full
            # 16.8 MB weight fill is ~47us). Weights and block-0 xg stay
            # SBUF-resident across reps.
            xg0sb = []
            for k in range(KD):
                t = xg0p.tile([P, BLK], bf16, tag=f"xg0_{k}")
                nc.sync.dma_start(t[:, :blocks[0]], xg[k, :, :blocks[0]])
                xg0sb.append(t)
            w1sb = [None] * KF
            w2sb = [None] * KD
            for fi in range(KF):
                t = w1p.tile([P, D], bf16, tag=f"w1_{fi}")
                nc.scalar.dma_start(t[:], w1[fi])
                w1sb[fi] = t
            for dd in range(KD):
                t = w2p.tile([P, F], bf16, tag=f"w2_{dd}")
                nc.scalar.dma_start(t[:], w2[dd])
                w2sb[dd] = t

            def body():
                for b, bw in enumerate(blocks):
                    c0 = b * BLK
                    if b == 0:
                        xgsb = xg0sb
                    else:
                        xgsb = []
                        for k in range(KD):
                            t = xgp.tile([P, BLK], bf16, tag=f"xg_{k}")
                            nc.sync.dma_start(t[:, :bw], xg[k, :, c0:c0 + bw])
                            xgsb.append(t)
                    hts = []
                    for fi in range(KF):
                        ps = ps1.tile([P, BLK], f32)
                        for k in range(KD):
                            nc.tensor.matmul(
                                ps[:, :bw], w1sb[fi][:, k * P:(k + 1) * P],
                                xgsb[k][:, :bw],
                                start=(k == 0), stop=(k == KD - 1))
                        ht = htp.tile([P, BLK], bf16, tag=f"ht_{fi}")
                        nc.vector.tensor_scalar_max(
                            ht[:, :bw], ps[:, :bw], 0.0)
                        hts.append(ht)
                    for d in range(KD):
                        ps_ = ps2.tile([P, BLK], f32)
                        for k2 in range(KF):
                            nc.tensor.matmul(
                                ps_[:, :bw],
                                w2sb[d][:, k2 * P:(k2 + 1) * P],
                                hts[k2][:, :bw],
                                start=(k2 == 0), stop=(k2 == KF - 1))
                        ob = outp.tile([P, BLK], bf16)
                        if d % 2 == 1:
                            nc.scalar.activation(
                                ob[:, :bw], ps_[:, :bw],
                                mybir.ActivationFunctionType.Copy)
                        else:
                            nc.vector.tensor_copy(ob[:, :bw], ps_[:, :bw])
                        nc.scalar.dma_start(ys[d, :, c0:c0 + bw], ob[:, :bw])

            if reps == 1:
                body()
            else:
                # Bench-only variant: repeat the whole body on-device so the
                # per-iteration time dominates host dispatch overhead.
                with tc.For_i(0, reps, 1):
                    body()
    nc.compile()
    return nc


def _get_compiled(cmax: int):
    reps = int(os.environ.get("KERNEL_REPS", "1"))
    key = (cmax, reps)
    if key not in _COMPILED:
        _COMPILED[key] = _build(cmax, reps)
    return _COMPILED[key]


def kernel(x, Wr, W1, W2, _timing=None):
    from concourse.bass_utils import run_bass_kernel_spmd

    x = np.asarray(x, dtype=np.float32)
    Wr = np.asarray(Wr, dtype=np.float32)
    W1 = np.asarray(W1, dtype=np.float32)
    W2 = np.asarray(W2, dtype=np.float32)
    xf = x.reshape(T, D)

    # --- Host router (replicated, reference-exact) => sharding plan ---
    idxs, gates = _route(xf, Wr)
    counts = [int(np.count_nonzero(gates[e])) for e in range(E)]
    cmax = min(CAP, max(P, max(math.ceil(c / P) * P for c in counts)))

    # --- Dispatch: gather routed rows per expert, fold gate, cast bf16 ---
    xfT = np.ascontiguousarray(xf.T)                       # [D, T] f32
    in_maps = []
    for e in range(E):
        n = counts[e]
        xge = np.zeros((D, cmax), dtype=np.float32)
        xge[:, :n] = xfT[:, idxs[e][:n]] * gates[e][:n][None, :]
        w1t = W1[e].reshape(KD, P, KF, P).transpose(2, 1, 0, 3)
        w2t = W2[e].reshape(KF, P, KD, P).transpose(2, 1, 0, 3)
        in_maps.append({
            "xg": np.ascontiguousarray(
                xge.reshape(KD, P, cmax)).astype(BF16),
            "w1": np.ascontiguousarray(w1t.reshape(KF, P, D)).astype(BF16),
            "w2": np.ascontiguousarray(w2t.reshape(KD, P, F)).astype(BF16),
        })

    # --- Device: 8 expert FFNs on 8 cores ---
    nc = _get_compiled(cmax)
    want_trace = _timing is not None and os.environ.get("KERNEL_TRACE", "1") == "1"
    tcores = [int(c) for c in os.environ.get(
        "KERNEL_TRACE_CORES", ",".join(map(str, range(NCORES)))).split(",")]
    try:
        res = run_bass_kernel_spmd(
            nc, in_maps, list(range(NCORES)),
            trace=want_trace,
            trace_cores=tcores if want_trace else None,
        )
    except ModuleNotFoundError:
        # NTFF profile hook unavailable in this environment: run untraced.
        res = run_bass_kernel_spmd(nc, in_maps, list(range(NCORES)))
    if _timing is not None:
        _timing["exec_time_ns"] = res.exec_time_ns
        _timing["results"] = res

    # --- Combine/unshard: scatter-add gated outputs into y (host, f32) ---
    y = np.zeros((T, D), dtype=np.float32)
    for e in range(E):
        n = counts[e]
        yse = res.results[e]["ys"]                         # [KD, P, cmax] bf16
        y[idxs[e][:n]] += yse.reshape(D, cmax)[:, :n].T.astype(np.float32)
    return y.reshape(B, S, D)
